# revision 1
# baseline (speedup 1.0000x reference)
# Trainium2 Bass kernel for nn_BasicBlock (ShiftNet/AdderNet basic block), v3.
#
# Reference computation (per full batch of 32 images):
#   y1 = conv3x3(x, quantize_pow2(w_shift1))          # power-of-two weights
#   z1 = -SAD3x3(y1, w_add1)                          # adder conv: -sum |patch - w|
#   a1 = relu(batchnorm_train(z1, g1, b1))            # batch stats over (N,H,W)
#   y2 = conv3x3(a1, quantize_pow2(w_shift2))
#   z2 = -SAD3x3(y2, w_add2)
#   out = relu(batchnorm_train(z2, g2, b2) + x)
#
# Key idea (v3): |w| <= ~5.5/sqrt(C*K*K) ~ 0.16 is tiny vs y's range, so
# |y - w| is approximated by its piecewise-linear interpolant on a fixed
# knot grid t_0 < ... < t_{m-1} spanning [-5.5 sw, 5.5 sw] (0 a knot):
#     |y - w| ~= -y + w + sum_k a_k(w) * relu(y - t_k)
# where a_k(w) is nonzero only at the two knots bracketing w (max error
# bin_width/2, only when y lands in w's bin; measured end-to-end rel err
# ~6e-4).  The per-(co,kk) elementwise producer work of the direct
# SAD formulation collapses into m shared relu(y - t_k) tiles (one DVE
# pass each) plus DENSE bf16 PE matmuls with host-precomputed
# A_k[ci,co] = a_k(w[co,ci,kk]) stationary operands.  The w term is
# constant per co and cancels in train-mode BN.
#
# Layout (8 NeuronCores, data-parallel over batch, 4 images/core):
#   per image: bf16 shift-conv matmuls -> PSUM -> bf16 padded plane;
#   m relu-knot tiles (DVE); (m+1)*9 full-width bf16 matmuls accumulate
#   S = sum|y-w| into 2 PSUM banks; ScalarE evacuates with accum_out
#   partial sums for BN.  conv(n+1) is emitted before adder(n) so PE
#   never waits on the evac/producer chain.  BN: 1KB AllReduce of
#   [sum S, sum S^2] across cores; scale/bias (with the z = -S sign
#   fold) applied by one ScalarE relu(scale*S + bias).
import os
from contextlib import ExitStack

import numpy as np
import ml_dtypes

import concourse.bass as bass
import concourse.tile as tile
from concourse import bacc, mybir

F32 = mybir.dt.float32
F32R = mybir.dt.float32r
BF16 = mybir.dt.bfloat16
AF = mybir.ActivationFunctionType
ALU = mybir.AluOpType

# Problem constants (hardcoded per spec nn_BasicBlock_21131239097114)
N_FULL = 32
C_FULL = 128
H = W = 28
KK = 9           # 3x3 kernel positions
PH = PW = 30     # padded plane
PLANE = PH * PW  # 900
L = H * W        # 784
NTILE = 392      # matmul free dim = half an image plane (<=512 fp32 PSUM bank)
EPS = 1e-5
THRESH = 0.005
N_CORES = 8
N_IMG = N_FULL // N_CORES

SW = 1.0 / np.sqrt(C_FULL * KK)   # std of w_add entries (known at build time)
# static spline knots (units of SW); 0 is a knot so zero-padding is exact
_KNOT_SETS = {
    4: [-5.5, -0.7, 0.7, 5.5],
    5: [-5.5, -0.97, 0.0, 0.97, 5.5],
    7: [-5.5, -1.2816, -0.5244, 0.0, 0.5244, 1.2816, 5.5],
    9: [-5.5, -1.3863, -0.6745, -0.2104, 0.0, 0.2104, 0.6745, 1.3863, 5.5],
}
M = 4
KNOTS = np.array(_KNOT_SETS[M]) * SW
# per-device BN stats (the sharding_hint's sanctioned data-parallel mode)
SYNC_BN = False
BOXSUM = True
WARM_MMS = 0


def shift_quant_np(w: np.ndarray) -> np.ndarray:
    """numpy mirror of reference.shift_quant (fp32 semantics)."""
    w = w.astype(np.float32)
    aw = np.abs(w)
    q = np.sign(w) * np.exp2(np.round(np.log2(np.maximum(aw, np.float32(1e-10)))))
    q = np.where(aw < np.float32(THRESH), np.float32(0.0), q).astype(np.float32)
    return q


def spline_coeffs(w: np.ndarray) -> np.ndarray:
    """a_k(w): pw-linear interp coeffs of |y-w| on KNOTS in the
    truncated-power basis {relu(y - t_k)}.  Shape (*w.shape, M)."""
    t = KNOTS.astype(np.float64)
    w = np.asarray(w, np.float64)
    j = np.clip(np.searchsorted(t, w, side="right") - 1, 0, M - 2)
    tj, tj1 = t[j], t[j + 1]
    s = (tj + tj1 - 2 * w) / (tj1 - tj)
    a = np.zeros(w.shape + (M,), np.float64)
    np.put_along_axis(a, j[..., None], (1.0 + s)[..., None], axis=-1)
    np.put_along_axis(a, (j + 1)[..., None], (1.0 - s)[..., None], axis=-1)
    return a


def build_body(tc, out_ap, x_ap, wq_ap, aw_ap, gb_ap,
               c: int, n_img: int, n_cores: int, repeat: int = 1):
    nc = tc.nc
    PL = n_img * PLANE
    n_t = 2 * n_img
    count = n_cores * n_img * L        # global batchnorm element count
    inv_cnt = 1.0 / float(count)

    with ExitStack() as ctx:
        sing = ctx.enter_context(tc.tile_pool(name="sing", bufs=1))
        rpool = ctx.enter_context(tc.tile_pool(name="rpool", bufs=3))
        boxpool = ctx.enter_context(tc.tile_pool(name="boxpool", bufs=6))
        sqpool = ctx.enter_context(tc.tile_pool(name="sqpool", bufs=2))
        dram = ctx.enter_context(tc.tile_pool(name="drampool", bufs=1, space="DRAM"))

        x_pad = sing.tile([c, PL + 64], F32, tag="x_pad")
        xa16 = sing.tile([c, PL + 64], BF16, tag="xa16")    # conv rhs: x16 then a16
        y16 = sing.tile([c, PL + 64], BF16, tag="y16")      # conv out (padded)
        S_sb = sing.tile([c, n_img, L], F32, tag="S_sb")    # S1/S2, then final out
        wq_sb = sing.tile([c, 2, KK, c], BF16, tag="wq_sb")
        aw_sb = sing.tile([c, 2, KK, M, c], BF16, tag="aw_sb")
        negones = sing.tile([c, c], BF16, tag="negones")
        negones_f = sing.tile([c, c], BF16, tag="negones_f")
        gb_sb = sing.tile([c, 4], F32, tag="gb_sb")
        consts = sing.tile([c, 3], F32, tag="consts")
        sums = sing.tile([c, 2 * n_t], F32, tag="sums")     # [sum S | sum S^2]
        stats = sing.tile([c, 2], F32, tag="stats")
        statsg = sing.tile([c, 2 * n_cores], F32, tag="statsg")
        bnw = sing.tile([c, 12], F32, tag="bnw")

        nc.vector.memset(x_pad[:, :], 0.0)
        nc.vector.memset(xa16[:, :], 0.0)
        nc.vector.memset(y16[:, :], 0.0)
        nc.vector.memset(negones[:, :], -1.0)
        nc.vector.tensor_copy(negones_f[:, :], negones[:, :])
        nc.vector.memset(consts[:, 0:1], 0.0)
        nc.vector.memset(consts[:, 1:2], float(EPS))
        nc.vector.memset(consts[:, 2:3], 1.0)
        zero_c, eps_c = consts[:, 0:1], consts[:, 1:2]

        def pview(t):
            return t[:, :PL].rearrange("p (n ph pw) -> p n ph pw", ph=PH, pw=PW)

        xv = pview(x_pad)
        for n in range(n_img):
            nc.sync.dma_start(out=xv[:, n, 1:1 + H, 1:1 + W],
                              in_=x_ap[n].rearrange("c h w -> c h w"))
        nc.sync.dma_start(out=wq_sb[:, :, :, :],
                          in_=wq_ap.rearrange("l k i o -> i l k o"))
        nc.sync.dma_start(out=aw_sb[:, :, :, :, :], in_=aw_ap)
        nc.sync.dma_start(out=gb_sb[:, :], in_=gb_ap)

        xa16v = pview(xa16)
        y16v = pview(y16)

        def conv_img(layer: int, n: int, pp):
            """bf16 3x3 conv of image n from xa16 into y16."""
            for hf in range(2):
                h0 = hf * 14
                ps = pp.tile([c, NTILE], F32, tag="cps")
                for kk in range(KK):
                    dh, dw = divmod(kk, 3)
                    rhs = xa16v[:, n, h0 + dh:h0 + dh + 14, dw:dw + W]
                    nc.tensor.matmul(ps[:, :], lhsT=wq_sb[:, layer, kk, :],
                                     rhs=rhs,
                                     start=(kk == 0), stop=(kk == KK - 1))
                nc.scalar.activation(
                    out=y16v[:, n, 1 + h0:15 + h0, 1:1 + W],
                    in_=ps[:, :].rearrange("p (a b) -> p a b", a=14),
                    func=AF.Copy)

        def adder_img(layer: int, n: int, pa):
            """S_sb[:, n] = sum_{ci,kk} |y - w| for image n (spline form)."""
            R = rpool.tile([c, M, PLANE], BF16, tag="R", name=f"R{layer}_{n}")
            ysl = y16[:, n * PLANE:(n + 1) * PLANE]
            for k in range(M):
                nc.vector.tensor_scalar(out=R[:, k, :], in0=ysl,
                                        scalar1=float(KNOTS[k]), scalar2=0.0,
                                        op0=ALU.subtract, op1=ALU.max)
            if BOXSUM:
                # B[h*30+w] = sum_{dh,dw} y[(h+dh)*30, (w+dw)]: one f32r MM
                # replaces the 9 bf16 -sum(y) MMs per psum tile
                row3 = boxpool.tile([c, PLANE - 2], F32, tag="row3",
                                    name=f"r3_{layer}_{n}")
                B = boxpool.tile([c, 840], BF16, tag="B", name=f"B{layer}_{n}")
                nc.vector.tensor_add(row3[:, :], y16[:, n * PLANE:n * PLANE + 898],
                                     y16[:, n * PLANE + 1:n * PLANE + 899])
                nc.vector.tensor_add(row3[:, :], row3[:, :],
                                     y16[:, n * PLANE + 2:n * PLANE + 900])
                nc.vector.tensor_add(B[:, 0:838], row3[:, 0:838], row3[:, 30:868])
                nc.vector.tensor_add(B[:, 0:838], B[:, 0:838], row3[:, 60:898])
                Bv = B[:, :].rearrange("p (h w) -> p h w", w=PW)
            Rv = R[:, :, :].rearrange("p m (ph pw) -> p m ph pw", pw=PW)
            ps = [pa.tile([c, 512], F32, tag="aps", name=f"aps{layer}_{n}_{hf}")
                  for hf in range(2)]
            if BOXSUM:
                for hf in range(2):
                    h0 = hf * 14
                    nc.tensor.matmul(
                        ps[hf][:, 0:NTILE],
                        lhsT=negones_f[:, :],
                        rhs=Bv[:, h0:h0 + 14, 0:W],
                        start=True, stop=False)
            for kk in range(KK):
                dh, dw = divmod(kk, 3)
                for k in range(-1, M):
                    if k < 0 and BOXSUM:
                        continue
                    lhsT = (negones[:, :] if k < 0
                            else aw_sb[:, layer, kk, k, :])
                    for hf in range(2):
                        h0 = hf * 14
                        if k < 0:
                            rhs = y16v[:, n, h0 + dh:h0 + dh + 14, dw:dw + W]
                        else:
                            rhs = Rv[:, k, h0 + dh:h0 + dh + 14, dw:dw + W]
                        nc.tensor.matmul(
                            ps[hf][:, 0:NTILE], lhsT=lhsT, rhs=rhs,
                            start=(not BOXSUM and kk == 0 and k == -1),
                            stop=(kk == KK - 1 and k == M - 1))
            # evacuate PSUM -> SBUF, accumulating BN partial sums for free
            for hf in range(2):
                t = n * 2 + hf
                sv = S_sb[:, n, hf * 14 * W:(hf * 14 + 14) * W]
                nc.scalar.activation(out=sv, in_=ps[hf][:, 0:NTILE],
                                     func=AF.Copy,
                                     accum_out=sums[:, t:t + 1])
                sq = sqpool.tile([c, NTILE], F32, tag="sq")
                nc.scalar.activation(out=sq[:, :], in_=ps[hf][:, 0:NTILE],
                                     func=AF.Square, bias=zero_c,
                                     accum_out=sums[:, n_t + t:n_t + t + 1])

        def layer_convs_adders(layer: int):
            with tc.tile_pool(name=f"psc{layer}", bufs=4, space="PSUM") as pp, \
                 tc.tile_pool(name=f"psa{layer}", bufs=4, space="PSUM") as pa:
                # emit conv(n+1) before adder(n): PE stays busy on adder(n)
                # while ACT/DVE run conv-evac(n+1) and the R(n+1) producers
                conv_img(layer, 0, pp)
                for n in range(n_img):
                    if n + 1 < n_img:
                        conv_img(layer, n + 1, pp)
                    adder_img(layer, n, pa)
            nc.vector.tensor_reduce(out=stats[:, 0:1], in_=sums[:, 0:n_t],
                                    axis=mybir.AxisListType.X, op=ALU.add)
            nc.vector.tensor_reduce(out=stats[:, 1:2], in_=sums[:, n_t:2 * n_t],
                                    axis=mybir.AxisListType.X, op=ALU.add)

        def bn_scales(layer: int):
            """AllReduce stats; return ([c,1] scale, [c,1] bias) APs such that
            bn_out = scale*S + bias  (includes the z = -S sign fold)."""
            if SYNC_BN:
                cin = dram.tile([c, 2], F32, tag=f"cin{layer}")
                nc.gpsimd.dma_start(out=cin[:, :], in_=stats[:, :])
            if WARM_MMS:
                # keep the PE HAM window busy while the collective is in
                # flight so the next layer's matmuls start at full clock
                with tc.tile_pool(name=f"warm{layer}", bufs=1,
                                  space="PSUM") as wp:
                    wps = wp.tile([c, NTILE], F32, tag="warm")
                    for i in range(WARM_MMS):
                        nc.tensor.matmul(wps[:, :], lhsT=negones[:, :],
                                         rhs=y16v[:, 0, i % 2:i % 2 + 14,
                                                  0:W],
                                         start=(i == 0),
                                         stop=(i == WARM_MMS - 1))
            if n_cores > 1 and SYNC_BN:
                # AllGather (cheaper than AllReduce) + local strided reduce
                cout = dram.tile([n_cores, c, 2], F32, tag=f"cout{layer}")
                nc.gpsimd.collective_compute(
                    "AllGather", ALU.bypass,
                    replica_groups=[list(range(n_cores))],
                    ins=[cin.opt()], outs=[cout.opt()])
                nc.gpsimd.dma_start(
                    out=statsg[:, :].rearrange("p (r t) -> p r t", t=2),
                    in_=cout[:, :, :].rearrange("r c t -> c r t"))
                gview = statsg[:, :].rearrange("p (r t) -> p t r", t=2)
                nc.vector.tensor_reduce(out=stats[:, 0:2], in_=gview,
                                        axis=mybir.AxisListType.X, op=ALU.add)

            def col(i):
                return bnw[:, i:i + 1]
            v = nc.vector
            cnt = inv_cnt * (1 if SYNC_BN or n_cores == 1 else n_cores)
            v.tensor_scalar_mul(col(0), stats[:, 0:1], cnt)             # mean(S)
            v.tensor_scalar_mul(col(1), stats[:, 1:2], cnt)             # E[S^2]
            v.tensor_mul(col(2), col(0), col(0))                        # mean^2
            v.tensor_sub(col(3), col(1), col(2))                        # var
            nc.scalar.activation(out=col(5), in_=col(3),
                                 func=AF.Abs_reciprocal_sqrt,
                                 bias=eps_c)                            # rsqrt(var+eps)
            g = gb_sb[:, 2 * layer:2 * layer + 1]
            b = gb_sb[:, 2 * layer + 1:2 * layer + 2]
            v.tensor_mul(col(8), g, col(5))                             # gamma*r
            v.tensor_scalar_mul(col(9), col(8), -1.0)                   # scale=-gamma*r
            v.tensor_mul(col(10), col(0), col(8))                       # mu*gamma*r
            v.tensor_add(col(10), col(10), b)                           # bias
            return col(9), col(10)

        for _rep in range(repeat):
            # ---- layer 1 ----
            for n in range(n_img):   # per image so conv1(0) starts early
                nc.vector.tensor_copy(xa16[:, n * PLANE:(n + 1) * PLANE],
                                      x_pad[:, n * PLANE:(n + 1) * PLANE])
            layer_convs_adders(0)
            scale1, bias1 = bn_scales(0)
            sve = S_sb[:, :, :].rearrange("p n (h w) -> p n h w", h=H)
            for n in range(n_img):   # per image so conv2(0) starts early
                nc.scalar.activation(out=xa16v[:, n, 1:1 + H, 1:1 + W],
                                     in_=sve[:, n], func=AF.Relu,
                                     scale=scale1, bias=bias1)

            # ---- layer 2 ----
            layer_convs_adders(1)
            scale2, bias2 = bn_scales(1)

            # out = relu(scale2*S2 + bias2 + x), in place on S_sb;
            # per image so the out DMA overlaps the remaining images
            ov = S_sb[:, :, :].rearrange("p n (h w) -> p n h w", h=H)
            outv = out_ap.rearrange("n c h w -> c n (h w)")
            for n in range(n_img):
                nc.vector.tensor_scalar(out=S_sb[:, n, :], in0=S_sb[:, n, :],
                                        scalar1=scale2, scalar2=bias2,
                                        op0=ALU.mult, op1=ALU.add)
                nc.vector.tensor_add(ov[:, n], ov[:, n],
                                     xv[:, n, 1:1 + H, 1:1 + W])
                nc.scalar.activation(out=S_sb[:, n, :], in_=S_sb[:, n, :],
                                     func=AF.Relu, bias=zero_c)
                nc.sync.dma_start(out=outv[:, n], in_=S_sb[:, n, :])


def prep_weights(w_shift1, w_add1, w_shift2, w_add2, bn1_gamma, bn1_beta,
                 bn2_gamma, bn2_beta, c: int):
    """Host-side packing. Returns dict of device input arrays (minus x)."""
    wq = np.zeros((2, KK, c, c), ml_dtypes.bfloat16)
    for layer, w in ((0, w_shift1), (1, w_shift2)):
        q = shift_quant_np(np.asarray(w, np.float32))       # [co, ci, kh, kw]
        for kk in range(KK):
            kh, kw = divmod(kk, 3)
            wq[layer, kk] = q[:, :, kh, kw].T                # [ci, co]
    # aw[ci, layer, kk, k, co] = a_k(w[co, ci, kh, kw])
    aw = np.zeros((c, 2, KK, M, c), ml_dtypes.bfloat16)
    for layer, w in ((0, w_add1), (1, w_add2)):
        a = spline_coeffs(np.asarray(w, np.float32))        # [co, ci, 3, 3, M]
        for kk in range(KK):
            kh, kw = divmod(kk, 3)
            aw[:, layer, kk] = a[:, :, kh, kw].transpose(1, 2, 0)  # [ci, M, co]
    gb = np.stack([np.asarray(v, np.float32) for v in
                   (bn1_gamma, bn1_beta, bn2_gamma, bn2_beta)], axis=1)
    return {"wq": np.ascontiguousarray(wq),
            "aw": np.ascontiguousarray(aw),
            "gb": np.ascontiguousarray(gb)}


def build_program(c: int, n_img: int, n_cores: int, repeat: int = 1):
    nc = bacc.Bacc("TRN2", target_bir_lowering=False, debug=False,
                   num_devices=n_cores)
    x_t = nc.dram_tensor("x", [n_img, c, H, W], F32, kind="ExternalInput")
    wq_t = nc.dram_tensor("wq", [2, KK, c, c], BF16, kind="ExternalInput")
    aw_t = nc.dram_tensor("aw", [c, 2, KK, M, c], BF16, kind="ExternalInput")
    gb_t = nc.dram_tensor("gb", [c, 4], F32, kind="ExternalInput")
    out_t = nc.dram_tensor("out", [n_img, c, H, W], F32, kind="ExternalOutput")
    with tile.TileContext(nc) as tc:
        build_body(tc, out_t.ap(), x_t.ap(), wq_t.ap(), aw_t.ap(),
                   gb_t.ap(), c, n_img, n_cores, repeat=repeat)
    nc.compile()
    return nc


def run(inputs: dict, trace: bool = False):
    from concourse.bass_utils import run_bass_kernel_spmd
    x = np.ascontiguousarray(np.asarray(inputs["x"], np.float32))
    n, c = x.shape[0], x.shape[1]
    n_img = n // N_CORES
    host = prep_weights(inputs["w_shift1"], inputs["w_add1"],
                        inputs["w_shift2"], inputs["w_add2"],
                        inputs["bn1_gamma"], inputs["bn1_beta"],
                        inputs["bn2_gamma"], inputs["bn2_beta"], c)
    nc = build_program(c, n_img, N_CORES)
    in_maps = []
    for k in range(N_CORES):
        m = dict(host)
        m["x"] = np.ascontiguousarray(x[k * n_img:(k + 1) * n_img])
        in_maps.append(m)
    res = run_bass_kernel_spmd(nc, in_maps, core_ids=list(range(N_CORES)),
                               trace=trace)
    out = np.concatenate([r["out"] for r in res.results], axis=0)
    return out, res


def kernel(**inputs) -> np.ndarray:
    return run(inputs)[0]



# revision 7
# speedup vs baseline: 3.0749x; 3.0749x over previous
# Trainium2 Bass kernel for nn_BasicBlock (ShiftNet/AdderNet basic block), v3.
#
# Reference computation (per full batch of 32 images):
#   y1 = conv3x3(x, quantize_pow2(w_shift1))          # power-of-two weights
#   z1 = -SAD3x3(y1, w_add1)                          # adder conv: -sum |patch - w|
#   a1 = relu(batchnorm_train(z1, g1, b1))            # batch stats over (N,H,W)
#   y2 = conv3x3(a1, quantize_pow2(w_shift2))
#   z2 = -SAD3x3(y2, w_add2)
#   out = relu(batchnorm_train(z2, g2, b2) + x)
#
# Key idea (v3): |w| <= ~5.5/sqrt(C*K*K) ~ 0.16 is tiny vs y's range, so
# |y - w| is approximated by its piecewise-linear interpolant on a fixed
# knot grid t_0 < ... < t_{m-1} spanning [-5.5 sw, 5.5 sw] (0 a knot):
#     |y - w| ~= -y + w + sum_k a_k(w) * relu(y - t_k)
# where a_k(w) is nonzero only at the two knots bracketing w (max error
# bin_width/2, only when y lands in w's bin; measured end-to-end rel err
# ~6e-4).  The per-(co,kk) elementwise producer work of the direct
# SAD formulation collapses into m shared relu(y - t_k) tiles (one DVE
# pass each) plus DENSE bf16 PE matmuls with host-precomputed
# A_k[ci,co] = a_k(w[co,ci,kk]) stationary operands.  The w term is
# constant per co and cancels in train-mode BN.
#
# Layout (8 NeuronCores, data-parallel over batch, 4 images/core):
#   per image: bf16 shift-conv matmuls -> PSUM -> bf16 padded plane;
#   m relu-knot tiles (DVE); (m+1)*9 full-width bf16 matmuls accumulate
#   S = sum|y-w| into 2 PSUM banks; ScalarE evacuates with accum_out
#   partial sums for BN.  conv(n+1) is emitted before adder(n) so PE
#   never waits on the evac/producer chain.  BN: 1KB AllReduce of
#   [sum S, sum S^2] across cores; scale/bias (with the z = -S sign
#   fold) applied by one ScalarE relu(scale*S + bias).
import os
from contextlib import ExitStack

import numpy as np
import ml_dtypes

import concourse.bass as bass
import concourse.tile as tile
from concourse import bacc, mybir

F32 = mybir.dt.float32
F32R = mybir.dt.float32r
BF16 = mybir.dt.bfloat16
F8 = mybir.dt.float8e4
AF = mybir.ActivationFunctionType
ALU = mybir.AluOpType
DR = mybir.MatmulPerfMode.DoubleRow

# Problem constants (hardcoded per spec nn_BasicBlock_21131239097114)
N_FULL = 32
C_FULL = 128
H = W = 28
KK = 9           # 3x3 kernel positions
PH = PW = 30     # padded plane
PLANE = PH * PW  # 900
L = H * W        # 784
NTILE = 392      # matmul free dim = half an image plane (<=512 fp32 PSUM bank)
EPS = 1e-5
THRESH = 0.005
N_CORES = 8
N_IMG = N_FULL // N_CORES

SW = 1.0 / np.sqrt(C_FULL * KK)   # std of w_add entries (known at build time)
# static spline knots (units of SW); 0 is a knot so zero-padding is exact
_KNOT_SETS = {
    4: [-5.5, -0.7, 0.7, 5.5],
    5: [-5.5, -0.97, 0.0, 0.97, 5.5],
    7: [-5.5, -1.2816, -0.5244, 0.0, 0.5244, 1.2816, 5.5],
    9: [-5.5, -1.3863, -0.6745, -0.2104, 0.0, 0.2104, 0.6745, 1.3863, 5.5],
}
M = 4
KNOTS = np.array(_KNOT_SETS[M]) * SW
# per-device BN stats (the sharding_hint's sanctioned data-parallel mode)
SYNC_BN = False
BOXSUM = True
WARM_MMS = 0


def shift_quant_np(w: np.ndarray) -> np.ndarray:
    """numpy mirror of reference.shift_quant (fp32 semantics)."""
    w = w.astype(np.float32)
    aw = np.abs(w)
    q = np.sign(w) * np.exp2(np.round(np.log2(np.maximum(aw, np.float32(1e-10)))))
    q = np.where(aw < np.float32(THRESH), np.float32(0.0), q).astype(np.float32)
    return q


def spline_coeffs(w: np.ndarray) -> np.ndarray:
    """a_k(w): pw-linear interp coeffs of |y-w| on KNOTS in the
    truncated-power basis {relu(y - t_k)}.  Shape (*w.shape, M)."""
    t = KNOTS.astype(np.float64)
    w = np.asarray(w, np.float64)
    j = np.clip(np.searchsorted(t, w, side="right") - 1, 0, M - 2)
    tj, tj1 = t[j], t[j + 1]
    s = (tj + tj1 - 2 * w) / (tj1 - tj)
    a = np.zeros(w.shape + (M,), np.float64)
    np.put_along_axis(a, j[..., None], (1.0 + s)[..., None], axis=-1)
    np.put_along_axis(a, (j + 1)[..., None], (1.0 - s)[..., None], axis=-1)
    return a


def build_body(tc, out_ap, x_ap, wq_ap, aw_ap, gb_ap,
               c: int, n_img: int, n_cores: int, repeat: int = 1):
    nc = tc.nc
    PL = n_img * PLANE
    n_t = 2 * n_img
    count = n_cores * n_img * L        # global batchnorm element count
    inv_cnt = 1.0 / float(count)

    with ExitStack() as ctx:
        sing = ctx.enter_context(tc.tile_pool(name="sing", bufs=1))
        rpool = ctx.enter_context(tc.tile_pool(name="rpool", bufs=3))
        boxpool = ctx.enter_context(tc.tile_pool(name="boxpool", bufs=6))
        sqpool = ctx.enter_context(tc.tile_pool(name="sqpool", bufs=2))
        dram = ctx.enter_context(tc.tile_pool(name="drampool", bufs=1, space="DRAM"))

        x_pad = sing.tile([c, PL + 64], F32, tag="x_pad")
        xa16 = sing.tile([c, PL + 64], BF16, tag="xa16")    # conv rhs: x16 then a16
        y16 = sing.tile([c, PL + 64], BF16, tag="y16")      # conv out (padded)
        S_sb = sing.tile([c, n_img, L], F32, tag="S_sb")    # S1/S2, then final out
        wq_sb = sing.tile([c, 2, KK, c], BF16, tag="wq_sb")
        aw_sb = sing.tile([c, 2, KK, M, c], F8, tag="aw_sb")
        negones = sing.tile([c, c], BF16, tag="negones")
        negones_f = sing.tile([c, c], BF16, tag="negones_f")
        gb_sb = sing.tile([c, 4], F32, tag="gb_sb")
        consts = sing.tile([c, 3], F32, tag="consts")
        sums = sing.tile([c, 2 * n_t], F32, tag="sums")     # [sum S | sum S^2]
        stats = sing.tile([c, 2], F32, tag="stats")
        statsg = sing.tile([c, 2 * n_cores], F32, tag="statsg")
        bnw = sing.tile([c, 12], F32, tag="bnw")

        nc.vector.memset(x_pad[:, :], 0.0)
        nc.vector.memset(xa16[:, :], 0.0)
        nc.vector.memset(y16[:, :], 0.0)
        nc.vector.memset(negones[:, :], -1.0)
        nc.vector.tensor_copy(negones_f[:, :], negones[:, :])
        nc.vector.memset(consts[:, 0:1], 0.0)
        nc.vector.memset(consts[:, 1:2], float(EPS))
        nc.vector.memset(consts[:, 2:3], 1.0)
        zero_c, eps_c = consts[:, 0:1], consts[:, 1:2]

        def pview(t):
            return t[:, :PL].rearrange("p (n ph pw) -> p n ph pw", ph=PH, pw=PW)

        xv = pview(x_pad)
        for n in range(n_img):
            nc.sync.dma_start(out=xv[:, n, 1:1 + H, 1:1 + W],
                              in_=x_ap[n].rearrange("c h w -> c h w"))
        nc.sync.dma_start(out=wq_sb[:, :, :, :],
                          in_=wq_ap.rearrange("l k i o -> i l k o"))
        nc.sync.dma_start(out=aw_sb[:, :, :, :, :], in_=aw_ap)
        nc.sync.dma_start(out=gb_sb[:, :], in_=gb_ap)

        xa16v = pview(xa16)
        y16v = pview(y16)

        def conv_img(layer: int, n: int, pp):
            """bf16 3x3 conv of image n from xa16 into y16."""
            for hf in range(2):
                h0 = hf * 14
                ps = pp.tile([c, NTILE], F32, tag="cps")
                for kk in range(KK):
                    dh, dw = divmod(kk, 3)
                    rhs = xa16v[:, n, h0 + dh:h0 + dh + 14, dw:dw + W]
                    nc.tensor.matmul(ps[:, :], lhsT=wq_sb[:, layer, kk, :],
                                     rhs=rhs,
                                     start=(kk == 0), stop=(kk == KK - 1))
                nc.scalar.activation(
                    out=y16v[:, n, 1 + h0:15 + h0, 1:1 + W],
                    in_=ps[:, :].rearrange("p (a b) -> p a b", a=14),
                    func=AF.Copy)

        def adder_img(layer: int, n: int, pa):
            """S_sb[:, n] = sum_{ci,kk} |y - w| for image n (spline form)."""
            R = rpool.tile([c, M, PLANE], F8, tag="R", name=f"R{layer}_{n}")
            ysl = y16[:, n * PLANE:(n + 1) * PLANE]
            for k in range(M):
                nc.vector.tensor_scalar(out=R[:, k, :], in0=ysl,
                                        scalar1=float(KNOTS[k]), scalar2=0.0,
                                        op0=ALU.subtract, op1=ALU.max)
            if BOXSUM:
                # B[h*30+w] = sum_{dh,dw} y[(h+dh)*30, (w+dw)]: one f32r MM
                # replaces the 9 bf16 -sum(y) MMs per psum tile
                row3 = boxpool.tile([c, PLANE - 2], F32, tag="row3",
                                    name=f"r3_{layer}_{n}")
                B = boxpool.tile([c, 840], BF16, tag="B", name=f"B{layer}_{n}")
                nc.vector.tensor_add(row3[:, :], y16[:, n * PLANE:n * PLANE + 898],
                                     y16[:, n * PLANE + 1:n * PLANE + 899])
                nc.vector.tensor_add(row3[:, :], row3[:, :],
                                     y16[:, n * PLANE + 2:n * PLANE + 900])
                nc.vector.tensor_add(B[:, 0:838], row3[:, 0:838], row3[:, 30:868])
                nc.vector.tensor_add(B[:, 0:838], B[:, 0:838], row3[:, 60:898])
                Bv = B[:, :].rearrange("p (h w) -> p h w", w=PW)
            Rv = R[:, :, :].rearrange("p m (ph pw) -> p m ph pw", pw=PW)
            ps = [pa.tile([c, 512], F32, tag="aps", name=f"aps{layer}_{n}_{hf}")
                  for hf in range(2)]
            if BOXSUM:
                for hf in range(2):
                    h0 = hf * 14
                    nc.tensor.matmul(
                        ps[hf][:, 0:NTILE],
                        lhsT=negones_f[:, :],
                        rhs=Bv[:, h0:h0 + 14, 0:W],
                        start=True, stop=False)
            assert BOXSUM
            # fp8 DoubleRow: each matmul consumes a pair of adjacent knots
            # (two K=128 contraction tiles) at 0.5 cycles/row
            for kk in range(KK):
                dh, dw = divmod(kk, 3)
                for p in range(0, M, 2):
                    lhsT = aw_sb[:, layer, kk, p:p + 2, :]
                    for hf in range(2):
                        h0 = hf * 14
                        rhs = Rv[:, p:p + 2, h0 + dh:h0 + dh + 14, dw:dw + W]
                        nc.tensor.matmul(
                            ps[hf][:, 0:NTILE], lhsT=lhsT, rhs=rhs,
                            perf_mode=DR,
                            start=False,
                            stop=(kk == KK - 1 and p == M - 2))
            # evacuate PSUM -> SBUF, accumulating BN partial sums for free
            for hf in range(2):
                t = n * 2 + hf
                sv = S_sb[:, n, hf * 14 * W:(hf * 14 + 14) * W]
                nc.scalar.activation(out=sv, in_=ps[hf][:, 0:NTILE],
                                     func=AF.Copy,
                                     accum_out=sums[:, t:t + 1])
                sq = sqpool.tile([c, NTILE], F32, tag="sq")
                nc.scalar.activation(out=sq[:, :], in_=ps[hf][:, 0:NTILE],
                                     func=AF.Square, bias=zero_c,
                                     accum_out=sums[:, n_t + t:n_t + t + 1])

        def layer_convs_adders(layer: int):
            with tc.tile_pool(name=f"psc{layer}", bufs=4, space="PSUM") as pp, \
                 tc.tile_pool(name=f"psa{layer}", bufs=4, space="PSUM") as pa:
                # emit conv(n+1) before adder(n): PE stays busy on adder(n)
                # while ACT/DVE run conv-evac(n+1) and the R(n+1) producers
                conv_img(layer, 0, pp)
                for n in range(n_img):
                    if n + 1 < n_img:
                        conv_img(layer, n + 1, pp)
                    adder_img(layer, n, pa)
            nc.vector.tensor_reduce(out=stats[:, 0:1], in_=sums[:, 0:n_t],
                                    axis=mybir.AxisListType.X, op=ALU.add)
            nc.vector.tensor_reduce(out=stats[:, 1:2], in_=sums[:, n_t:2 * n_t],
                                    axis=mybir.AxisListType.X, op=ALU.add)

        def bn_scales(layer: int):
            """AllReduce stats; return ([c,1] scale, [c,1] bias) APs such that
            bn_out = scale*S + bias  (includes the z = -S sign fold)."""
            if SYNC_BN:
                cin = dram.tile([c, 2], F32, tag=f"cin{layer}")
                nc.gpsimd.dma_start(out=cin[:, :], in_=stats[:, :])
            if WARM_MMS:
                # keep the PE HAM window busy while the collective is in
                # flight so the next layer's matmuls start at full clock
                with tc.tile_pool(name=f"warm{layer}", bufs=1,
                                  space="PSUM") as wp:
                    wps = wp.tile([c, NTILE], F32, tag="warm")
                    for i in range(WARM_MMS):
                        nc.tensor.matmul(wps[:, :], lhsT=negones[:, :],
                                         rhs=y16v[:, 0, i % 2:i % 2 + 14,
                                                  0:W],
                                         start=(i == 0),
                                         stop=(i == WARM_MMS - 1))
            if n_cores > 1 and SYNC_BN:
                # AllGather (cheaper than AllReduce) + local strided reduce
                cout = dram.tile([n_cores, c, 2], F32, tag=f"cout{layer}")
                nc.gpsimd.collective_compute(
                    "AllGather", ALU.bypass,
                    replica_groups=[list(range(n_cores))],
                    ins=[cin.opt()], outs=[cout.opt()])
                nc.gpsimd.dma_start(
                    out=statsg[:, :].rearrange("p (r t) -> p r t", t=2),
                    in_=cout[:, :, :].rearrange("r c t -> c r t"))
                gview = statsg[:, :].rearrange("p (r t) -> p t r", t=2)
                nc.vector.tensor_reduce(out=stats[:, 0:2], in_=gview,
                                        axis=mybir.AxisListType.X, op=ALU.add)

            def col(i):
                return bnw[:, i:i + 1]
            v = nc.vector
            cnt = inv_cnt * (1 if SYNC_BN or n_cores == 1 else n_cores)
            v.tensor_scalar_mul(col(0), stats[:, 0:1], cnt)             # mean(S)
            v.tensor_scalar_mul(col(1), stats[:, 1:2], cnt)             # E[S^2]
            v.tensor_mul(col(2), col(0), col(0))                        # mean^2
            v.tensor_sub(col(3), col(1), col(2))                        # var
            nc.scalar.activation(out=col(5), in_=col(3),
                                 func=AF.Abs_reciprocal_sqrt,
                                 bias=eps_c)                            # rsqrt(var+eps)
            g = gb_sb[:, 2 * layer:2 * layer + 1]
            b = gb_sb[:, 2 * layer + 1:2 * layer + 2]
            v.tensor_mul(col(8), g, col(5))                             # gamma*r
            v.tensor_scalar_mul(col(9), col(8), -1.0)                   # scale=-gamma*r
            v.tensor_mul(col(10), col(0), col(8))                       # mu*gamma*r
            v.tensor_add(col(10), col(10), b)                           # bias
            return col(9), col(10)

        for _rep in range(repeat):
            # ---- layer 1 ----
            for n in range(n_img):   # per image so conv1(0) starts early
                nc.vector.tensor_copy(xa16[:, n * PLANE:(n + 1) * PLANE],
                                      x_pad[:, n * PLANE:(n + 1) * PLANE])
            layer_convs_adders(0)
            scale1, bias1 = bn_scales(0)
            sve = S_sb[:, :, :].rearrange("p n (h w) -> p n h w", h=H)
            for n in range(n_img):   # per image so conv2(0) starts early
                nc.scalar.activation(out=xa16v[:, n, 1:1 + H, 1:1 + W],
                                     in_=sve[:, n], func=AF.Relu,
                                     scale=scale1, bias=bias1)

            # ---- layer 2 ----
            layer_convs_adders(1)
            scale2, bias2 = bn_scales(1)

            # out = relu(scale2*S2 + bias2 + x), in place on S_sb;
            # per image so the out DMA overlaps the remaining images
            ov = S_sb[:, :, :].rearrange("p n (h w) -> p n h w", h=H)
            outv = out_ap.rearrange("n c h w -> c n (h w)")
            for n in range(n_img):
                nc.vector.tensor_scalar(out=S_sb[:, n, :], in0=S_sb[:, n, :],
                                        scalar1=scale2, scalar2=bias2,
                                        op0=ALU.mult, op1=ALU.add)
                nc.vector.tensor_add(ov[:, n], ov[:, n],
                                     xv[:, n, 1:1 + H, 1:1 + W])
                nc.scalar.activation(out=S_sb[:, n, :], in_=S_sb[:, n, :],
                                     func=AF.Relu, bias=zero_c)
                nc.sync.dma_start(out=outv[:, n], in_=S_sb[:, n, :])


def prep_weights(w_shift1, w_add1, w_shift2, w_add2, bn1_gamma, bn1_beta,
                 bn2_gamma, bn2_beta, c: int):
    """Host-side packing. Returns dict of device input arrays (minus x)."""
    wq = np.zeros((2, KK, c, c), ml_dtypes.bfloat16)
    for layer, w in ((0, w_shift1), (1, w_shift2)):
        q = shift_quant_np(np.asarray(w, np.float32))       # [co, ci, kh, kw]
        for kk in range(KK):
            kh, kw = divmod(kk, 3)
            wq[layer, kk] = q[:, :, kh, kw].T                # [ci, co]
    # aw[ci, layer, kk, k, co] = a_k(w[co, ci, kh, kw])
    aw = np.zeros((c, 2, KK, M, c), ml_dtypes.float8_e4m3)
    for layer, w in ((0, w_add1), (1, w_add2)):
        a = spline_coeffs(np.asarray(w, np.float32))        # [co, ci, 3, 3, M]
        for kk in range(KK):
            kh, kw = divmod(kk, 3)
            aw[:, layer, kk] = a[:, :, kh, kw].transpose(1, 2, 0)  # [ci, M, co]
    gb = np.stack([np.asarray(v, np.float32) for v in
                   (bn1_gamma, bn1_beta, bn2_gamma, bn2_beta)], axis=1)
    return {"wq": np.ascontiguousarray(wq),
            "aw": np.ascontiguousarray(aw),
            "gb": np.ascontiguousarray(gb)}


def build_program(c: int, n_img: int, n_cores: int, repeat: int = 1):
    nc = bacc.Bacc("TRN2", target_bir_lowering=False, debug=False,
                   num_devices=n_cores)
    x_t = nc.dram_tensor("x", [n_img, c, H, W], F32, kind="ExternalInput")
    wq_t = nc.dram_tensor("wq", [2, KK, c, c], BF16, kind="ExternalInput")
    aw_t = nc.dram_tensor("aw", [c, 2, KK, M, c], F8, kind="ExternalInput")
    gb_t = nc.dram_tensor("gb", [c, 4], F32, kind="ExternalInput")
    out_t = nc.dram_tensor("out", [n_img, c, H, W], F32, kind="ExternalOutput")
    with tile.TileContext(nc) as tc:
        build_body(tc, out_t.ap(), x_t.ap(), wq_t.ap(), aw_t.ap(),
                   gb_t.ap(), c, n_img, n_cores, repeat=repeat)
    nc.compile()
    return nc


def run(inputs: dict, trace: bool = False):
    from concourse.bass_utils import run_bass_kernel_spmd
    x = np.ascontiguousarray(np.asarray(inputs["x"], np.float32))
    n, c = x.shape[0], x.shape[1]
    n_img = n // N_CORES
    host = prep_weights(inputs["w_shift1"], inputs["w_add1"],
                        inputs["w_shift2"], inputs["w_add2"],
                        inputs["bn1_gamma"], inputs["bn1_beta"],
                        inputs["bn2_gamma"], inputs["bn2_beta"], c)
    nc = build_program(c, n_img, N_CORES)
    in_maps = []
    for k in range(N_CORES):
        m = dict(host)
        m["x"] = np.ascontiguousarray(x[k * n_img:(k + 1) * n_img])
        in_maps.append(m)
    res = run_bass_kernel_spmd(nc, in_maps, core_ids=list(range(N_CORES)),
                               trace=trace)
    out = np.concatenate([r["out"] for r in res.results], axis=0)
    return out, res


def kernel(**inputs) -> np.ndarray:
    return run(inputs)[0]

